# revision 8
# baseline (speedup 1.0000x reference)
"""Bass/Trainium2 kernel for nn_Epdiff: Hermitian-truncated EPDiff smoothing
filters.

reference:
    cc(g) = -2*cos(2*pi*g) + 2
    coeff_sum[i,j,k] = cc(gx)[i] + cc(gy)[j] + cc(gz)[k]      (gx,gy 2m-band, gz m)
    val = (3*coeff_sum + 1)**6                                [2m, 2m, m]
    res_smooth = 1/val, res_sharp = val, broadcast to [B, 1, 2m, 2m, m]

Strategy (8 cores, batch-sharded): every core computes the full [128, 8192]
filter plane (partition axis = x, free axis = y*64+z) and writes its 4-batch
shard of both outputs.  The harness gate is rel_err < 2e-2, so outputs are
stored bf16 (<=2^-9 rounding) and upcast to f32 on the host: HBM write
traffic halves vs f32 (16.8 MB/core), which is the memory-regime bottleneck
(HBM-per-NC ~358 GB/s -> ~47 us floor).  Chunked along the free dim so
compute pipelines under the write stream:
  - DMA partition-broadcast of byz = cc(gy) (+) cc(gz)  into SBUF chunks
  - ACT:  v2 = Square(3*byz + bias_x)   with bias_x = 3*cc(gx)+1  per-partition
          nl = Ln(v2) ; rc = Exp(-3*nl) -> bf16   (1/v2^3 via the exp/ln
          table; seed err ~1e-4, far below the bf16 rounding already taken)
  - DVE:  v4 = v2*v2 ; v6 = v4*v2 -> bf16         (x**6 by repeated squaring)
  - DMA:  v6 -> sharp[b], rc -> smooth[b]  for each local batch b
"""

import os
import numpy as np

# ---- problem constants (hardcoded per spec) ----
MODE = 64
TWO_M = 2 * MODE            # 128 partitions
FREE = TWO_M * MODE         # 8192 = y*z free dim
BATCH = 32
N_CORES = 8
B_LOC = BATCH // N_CORES    # 4
# ramped chunk sizes: small first chunks get the first output DMA issued
# earlier (pipeline-fill latency), big tail chunks amortize op count
CHUNKS = [512, 1536, 2048, 4096]
assert sum(CHUNKS) == FREE
ALPHA = 3.0
GAMMA = 1.0

_NC = None                  # compiled Bass module, cached per process
LAST_RESULTS = None         # BassKernelResults of the most recent run (for test.py)

# "pe"   = raw Bass + PE outer-product broadcast (no SWDGE fill traffic)
# "raw"  = hand-scheduled raw Bass with DMA-broadcast fills
# "tile" = TileContext version
IMPL = os.environ.get("KERNEL_IMPL", "pe")

# v3 chunking: 512-col PSUM-bank granularity for PE->ACT, output chunks
# ramp up then stay at 2048 (4KB bf16 descriptor rows)
CHUNKS_PE = [512, 1536, 2048, 2048, 2048]
assert sum(CHUNKS_PE) == FREE and all(c % 512 == 0 for c in CHUNKS_PE)


def _ensure_path():
    try:
        import concourse.bass  # noqa: F401
        return
    except ImportError:
        pass
    import sys
    for p in ("/opt/trn_rl_repo", "/root/.axon_site/_ro/trn_rl_repo"):
        if os.path.isdir(p) and p not in sys.path:
            sys.path.insert(0, p)


def _legalize_single_wait(nc):
    """This container's walrus build rejects any instruction carrying more
    than one semaphore wait ("Too many sync wait commands"), including the
    Tile-generated kernel-tail Drain.  Split every multi-wait instruction
    into a chain of single-wait NoOps on the same engine followed by the
    original instruction with its last wait.  (NoOp, not Drain: a Drain
    would block on the engine's whole HWDGE queue, serializing in-flight
    DMAs when used mid-stream.)"""
    from concourse import mybir

    n_new = 0
    for fn in nc.m.functions:
        for bb in fn.blocks:
            insts = bb.instructions
            idx = 0
            while idx < len(insts):
                inst = insts[idx]
                si = inst.sync_info
                if si is not None and len(si.on_wait) > 1:
                    waits = list(si.on_wait)
                    eng = inst.engine
                    for k, w in enumerate(waits[:-1]):
                        d = mybir.InstNoOp(name=f"{inst.name}-sw{k}")
                        d.sync_info = mybir.SyncInfo(on_wait=[w], on_update=[])
                        d.engine = eng
                        insts.insert(idx, d)
                        idx += 1
                        n_new += 1
                    inst.sync_info = mybir.SyncInfo(
                        on_wait=[waits[-1]], on_update=list(si.on_update)
                    )
                idx += 1
    return n_new


def _build_nc(legalize=True):
    from concourse import bass, mybir
    import concourse.tile as tile

    f32 = mybir.dt.float32
    bf16 = mybir.dt.bfloat16
    nc = bass.Bass()

    byz = nc.dram_tensor("byz", [FREE], f32, kind="ExternalInput")
    biasx = nc.dram_tensor("biasx", [TWO_M], f32, kind="ExternalInput")
    sharp = nc.dram_tensor("sharp", [B_LOC, TWO_M, FREE], bf16, kind="ExternalOutput")
    smooth = nc.dram_tensor("smooth", [B_LOC, TWO_M, FREE], bf16, kind="ExternalOutput")
    with tile.TileContext(nc) as tc:
        with (
            tc.tile_pool(name="const", bufs=1) as cpool,
            tc.tile_pool(name="work", bufs=1) as wpool,
        ):
            bias_t = cpool.tile([TWO_M, 1], f32)
            nc.gpsimd.dma_start(bias_t[:], biasx[:, None])
            # TRN2 instructions take at most ONE sem wait; touch bias_t on
            # the scalar engine now so the chunk-0 activation doesn't need a
            # second wait for it on top of its bt-fill wait.
            bias_obs = cpool.tile([TWO_M, 1], f32)
            nc.scalar.copy(bias_obs[:], bias_t[:])

            off = 0
            for i, ch in enumerate(CHUNKS):
                sl = slice(off, off + ch)
                off += ch
                # Every tile gets a per-chunk tag (bufs=1, used exactly once)
                # so no slot is ever reused -> no WAR wait can pair up with a
                # RAW/DMA wait on any instruction (one-wait-per-inst limit).
                # partition-broadcast byz chunk into all 128 rows (SWDGE on
                # gpsimd: issuing fills from the scalar ring serializes them
                # behind the chunk activations and stretches the fill stream)
                bt = wpool.tile([TWO_M, ch], f32, tag=f"bt{i}")
                nc.gpsimd.dma_start(bt[:], byz[None, sl].broadcast_to((TWO_M, ch)))

                # v2 = (3*byz + (3*cc(gx)+1))^2 in one ACT op on the
                # otherwise-idle scalar engine
                v2 = wpool.tile([TWO_M, ch], f32, tag=f"v2{i}")
                nc.scalar.activation(
                    v2[:], bt[:], mybir.ActivationFunctionType.Square,
                    bias=bias_t[:, 0:1], scale=ALPHA,
                )
                # reciprocal via the ACT exp/ln table (square/ln/exp share
                # one table -> no reload): rc = exp(-3*ln(v2)) = 1/v2^3,
                # cast to bf16 at write.
                nl = wpool.tile([TWO_M, ch], f32, tag=f"nl{i}")
                nc.scalar.activation(
                    nl[:], v2[:], mybir.ActivationFunctionType.Ln
                )
                rc = wpool.tile([TWO_M, ch], bf16, tag=f"rc{i}")
                nc.scalar.activation(
                    rc[:], nl[:], mybir.ActivationFunctionType.Exp, scale=-3.0
                )

                # v6 = v2^3 on DVE, bf16 at the final write
                v4 = wpool.tile([TWO_M, ch], f32, tag=f"v4{i}")
                nc.vector.tensor_mul(v4[:], v2[:], v2[:])
                v6 = wpool.tile([TWO_M, ch], bf16, tag=f"v6{i}")
                nc.vector.tensor_mul(v6[:], v4[:], v2[:])

                # per-batch output DMAs, one contiguous HBM region each, all
                # on the SP HWDGE ring.  Queue-slot second waits on these
                # DMAs are split into NoOps by _legalize_single_wait.
                for b in range(B_LOC):
                    nc.sync.dma_start(sharp[b, :, sl], v6[:])
                for b in range(B_LOC):
                    nc.sync.dma_start(smooth[b, :, sl], rc[:])

    if legalize:
        _legalize_single_wait(nc)
    return nc


def _build_nc_raw():
    """Hand-scheduled raw-Bass variant: same dataflow as the Tile version but
    with manual semaphores (exactly one wait per instruction, satisfying this
    walrus build's limit) and none of TileContext's ~7.6us EVSEM preamble or
    ~2us drain/barrier tail.  Dependency DAG between engines is acyclic:
    gpsimd(fills) -> scalar(square/ln/exp) -> {vector(cube), sync(writes)}.
    No SBUF tile is ever reused, so there are no WAR hazards at all."""
    from contextlib import ExitStack
    from concourse import bass, mybir

    f32 = mybir.dt.float32
    bf16 = mybir.dt.bfloat16
    AF = mybir.ActivationFunctionType
    nc = bass.Bass()

    byz = nc.dram_tensor("byz", [FREE], f32, kind="ExternalInput")
    biasx = nc.dram_tensor("biasx", [TWO_M], f32, kind="ExternalInput")
    sharp = nc.dram_tensor("sharp", [B_LOC, TWO_M, FREE], bf16, kind="ExternalOutput")
    smooth = nc.dram_tensor("smooth", [B_LOC, TWO_M, FREE], bf16, kind="ExternalOutput")

    ctx = ExitStack()
    with ctx:
        # One sem per fill DMA: a shared counter is ambiguous because each
        # DMA's 16 per-engine sub-increments interleave with other in-flight
        # DMAs' (CoreSim's race detector rejects it).
        sb = ctx.enter_context(nc.semaphore("sb"))   # bias DMA
        sf = [
            ctx.enter_context(nc.semaphore(f"sf{i}")) for i in range(len(CHUNKS))
        ]
        sa = ctx.enter_context(nc.semaphore("sa"))   # ACT op completions
        sv = ctx.enter_context(nc.semaphore("sv"))   # DVE op completions
        ss = ctx.enter_context(nc.semaphore("ss"))   # sync output DMAs

        bias_t = ctx.enter_context(nc.sbuf_tensor("bias_t", [TWO_M, 1], f32))
        bias_o = ctx.enter_context(nc.sbuf_tensor("bias_o", [TWO_M, 1], f32))
        tiles = []
        for i, ch in enumerate(CHUNKS):
            t = {
                name: ctx.enter_context(
                    nc.sbuf_tensor(f"{name}{i}", [TWO_M, ch], f32)
                )
                for name in ("bt", "v2", "nl", "v4")
            }
            for name in ("v6", "rc"):
                t[name] = ctx.enter_context(
                    nc.sbuf_tensor(f"{name}{i}", [TWO_M, ch], bf16)
                )
            tiles.append(t)

        # ---- gpsimd: bias + per-chunk partition-broadcast fills (no waits)
        nc.gpsimd.dma_start(bias_t[:], biasx[:, None]).then_inc(sb, 16)
        off = 0
        for i, ch in enumerate(CHUNKS):
            t = tiles[i]
            nc.gpsimd.dma_start(
                t["bt"][:], byz[None, off:off + ch].broadcast_to((TWO_M, ch))
            ).then_inc(sf[i], 16)
            off += ch

        # ---- scalar (ACT): square + ln + exp; one wait per inst.
        # Observe the bias DMA once (wait propagation through the engine's
        # program order covers all later bias_t reads); same-engine RAW
        # (sq->ln->exp) needs explicit sa waits — engines pipeline, and the
        # race model demands a sem edge even within one engine.
        # ACT ticks: bias_o=1, then per chunk sq=3i+2, ln=3i+3, exp=3i+4.
        nc.scalar.copy(bias_o[:], bias_t[:])._wait_ge(sb, 16).then_inc(sa, 1)
        for i, ch in enumerate(CHUNKS):
            t = tiles[i]
            nc.scalar.activation(
                t["v2"][:], t["bt"][:], AF.Square,
                bias=bias_t[:, 0:1], scale=ALPHA,
            )._wait_ge(sf[i], 16).then_inc(sa, 1)
            nc.scalar.activation(t["nl"][:], t["v2"][:], AF.Ln)._wait_ge(
                sa, 3 * i + 2
            ).then_inc(sa, 1)
            # rc = exp(-3*ln(v2)) = 1/v2^3, cast to bf16 at write
            nc.scalar.activation(
                t["rc"][:], t["nl"][:], AF.Exp, scale=-3.0
            )._wait_ge(sa, 3 * i + 3).then_inc(sa, 1)

        # ---- vector (DVE): cube, bf16 at the final write.
        # DVE ticks: per chunk v4=2i+1, v6=2i+2.
        for i, ch in enumerate(CHUNKS):
            t = tiles[i]
            nc.vector.tensor_mul(t["v4"][:], t["v2"][:], t["v2"][:])._wait_ge(
                sa, 3 * i + 2
            ).then_inc(sv, 1)
            nc.vector.tensor_mul(t["v6"][:], t["v4"][:], t["v2"][:])._wait_ge(
                sv, 2 * i + 1
            ).then_inc(sv, 1)

        # ---- sync (SP): per-batch output writes
        off = 0
        for i, ch in enumerate(CHUNKS):
            t = tiles[i]
            sl = slice(off, off + ch)
            off += ch
            first = nc.sync.dma_start(sharp[0, :, sl], t["v6"][:])
            first._wait_ge(sv, 2 * i + 2)
            first.then_inc(ss, 16)
            for b in range(1, B_LOC):
                nc.sync.dma_start(sharp[b, :, sl], t["v6"][:]).then_inc(ss, 16)
            first = nc.sync.dma_start(smooth[0, :, sl], t["rc"][:])
            first._wait_ge(sa, 3 * i + 4)
            first.then_inc(ss, 16)
            for b in range(1, B_LOC):
                nc.sync.dma_start(smooth[b, :, sl], t["rc"][:]).then_inc(ss, 16)
        # retire: all output DMAs complete
        nc.sync.wait_ge(ss, 16 * 2 * B_LOC * len(CHUNKS))
    return nc


def _build_nc_pe():
    """Raw Bass, fills eliminated: the [128, free] broadcast of byz is built
    by the (otherwise idle) PE as a K=1 outer product ones[1,128]^T @
    (3*byz)[1,N] into PSUM, 512 cols per bank; ACT squares straight out of
    PSUM with the per-partition bias.  Inputs shrink from 4.2 MB of SWDGE
    broadcast traffic (which starved the HWDGE output stream while active)
    to ~50 KB, and the early input loads warm both HWDGE rings.  smooth
    writes go out on the now-idle gpsimd SWDGE ring so the two output
    streams issue descriptors in parallel.

    Engine DAG: {scalar,sync loads} -> PE(mm) -> ACT(square->ln->exp)
    -> {DVE(cube) -> sync(sharp)} / {gpsimd(smooth)}."""
    from contextlib import ExitStack
    from concourse import bass, mybir

    f32 = mybir.dt.float32
    f16 = mybir.dt.float16
    bf16 = mybir.dt.bfloat16
    AF = mybir.ActivationFunctionType
    nc = bass.Bass()

    rhs3 = nc.dram_tensor("rhs3", [1, FREE], f16, kind="ExternalInput")    # 3*byz
    ones1 = nc.dram_tensor("ones1", [1, TWO_M], f16, kind="ExternalInput")
    biasx = nc.dram_tensor("biasx", [TWO_M], f32, kind="ExternalInput")
    sharp = nc.dram_tensor("sharp", [B_LOC, TWO_M, FREE], bf16, kind="ExternalOutput")
    smooth = nc.dram_tensor("smooth", [B_LOC, TWO_M, FREE], bf16, kind="ExternalOutput")

    subs = [c // 512 for c in CHUNKS_PE]   # 512-col matmuls per chunk

    ctx = ExitStack()
    with ctx:
        sb = ctx.enter_context(nc.semaphore("sb"))    # bias DMA
        slh = ctx.enter_context(nc.semaphore("slh"))  # lhsT (ones) DMA
        sr = ctx.enter_context(nc.semaphore("sr"))    # rhs DMA
        sp = ctx.enter_context(nc.semaphore("sp"))    # PE matmul completions
        sa = ctx.enter_context(nc.semaphore("sa"))    # ACT op completions
        sv = ctx.enter_context(nc.semaphore("sv"))    # DVE op completions
        ss = ctx.enter_context(nc.semaphore("ss"))    # sync (sharp) DMAs
        sg = ctx.enter_context(nc.semaphore("sg"))    # gpsimd (smooth) DMAs

        bias_t = ctx.enter_context(nc.sbuf_tensor("bias_t", [TWO_M, 1], f32))
        bias_o = ctx.enter_context(nc.sbuf_tensor("bias_o", [TWO_M, 1], f32))
        lhsT_t = ctx.enter_context(nc.sbuf_tensor("lhsT_t", [1, TWO_M], f16))
        rhs_t = ctx.enter_context(nc.sbuf_tensor("rhs_t", [1, FREE], f16))
        # two 4-bank PSUM halves, cycled k%8 across the 16 512-col matmuls
        psA = ctx.enter_context(nc.psum_tensor("psA", [TWO_M, 2048], f32))
        psB = ctx.enter_context(nc.psum_tensor("psB", [TWO_M, 2048], f32))

        def psum_slice(k):
            half = psA if (k % 8) < 4 else psB
            j = k % 4
            return half[:, 512 * j:512 * (j + 1)]

        tiles = []
        for i, ch in enumerate(CHUNKS_PE):
            t = {
                name: ctx.enter_context(
                    nc.sbuf_tensor(f"{name}{i}", [TWO_M, ch], f32)
                )
                for name in ("v2", "nl", "v4")
            }
            for name in ("v6", "rc"):
                t[name] = ctx.enter_context(
                    nc.sbuf_tensor(f"{name}{i}", [TWO_M, ch], bf16)
                )
            tiles.append(t)

        # ---- input loads: bias + ones on the scalar HWDGE ring, rhs on the
        # sync HWDGE ring (doubles as the ring warm-up for the sharp stream)
        nc.scalar.dma_start(bias_t[:], biasx[:, None]).then_inc(sb, 16)
        nc.scalar.dma_start(lhsT_t[:], ones1[:, :]).then_inc(slh, 16)
        nc.sync.dma_start(rhs_t[:], rhs3[:, :]).then_inc(sr, 16)

        # ---- PE: 16 512-col outer products, bank = k % 8.
        # PE ticks: mm_k = k+1.  k>=8 reuses a bank -> WAR wait on the
        # square that consumed it (recorded below; ACT program order makes
        # sq ticks monotone in k).
        sq_tick = {}   # filled lazily; PE program emitted after ACT? no --
        # need sq ticks first, so precompute the ACT tick numbering:
        #   tick 1 = bias_obs, then per chunk: one square per sub, then
        #   ln, exp.
        tick = 1
        exp_tick = {}
        k = 0
        for c, ch in enumerate(CHUNKS_PE):
            for _ in range(subs[c]):
                tick += 1
                sq_tick[k] = tick
                k += 1
            exp_tick[c] = tick + 2
            tick += 2

        nc.tensor.wait_ge(slh, 16)   # spacer: stationary loaded
        k = 0
        for c, ch in enumerate(CHUNKS_PE):
            for _ in range(subs[c]):
                mm = nc.tensor.matmul(
                    psum_slice(k), lhsT_t[:, :], rhs_t[:, 512 * k:512 * (k + 1)],
                    start=True, stop=True,
                )
                if k == 0:
                    mm._wait_ge(sr, 16)
                elif k >= 8:
                    mm._wait_ge(sa, sq_tick[k - 8])
                mm.then_inc(sp, 1)
                k += 1

        # ---- scalar (ACT): bias observe, then per chunk: squares out of
        # PSUM (one per 512-col bank), ln, exp.  Square_k waits only on its
        # matmul (PSUM RAW); ln/exp wait on the same-engine RAW tick.
        nc.scalar.copy(bias_o[:], bias_t[:])._wait_ge(sb, 16).then_inc(sa, 1)
        k = 0
        for c, ch in enumerate(CHUNKS_PE):
            t = tiles[c]
            for j in range(subs[c]):
                nc.scalar.activation(
                    t["v2"][:, 512 * j:512 * (j + 1)], psum_slice(k), AF.Square,
                    bias=bias_t[:, 0:1],
                )._wait_ge(sp, k + 1).then_inc(sa, 1)
                k += 1
            nc.scalar.activation(t["nl"][:], t["v2"][:], AF.Ln)._wait_ge(
                sa, sq_tick[k - 1]
            ).then_inc(sa, 1)
            nc.scalar.activation(
                t["rc"][:], t["nl"][:], AF.Exp, scale=-3.0
            )._wait_ge(sa, sq_tick[k - 1] + 1).then_inc(sa, 1)

        # ---- vector (DVE): cube per chunk; v4 = 2c+1, v6 = 2c+2
        k = 0
        for c, ch in enumerate(CHUNKS_PE):
            t = tiles[c]
            k += subs[c]
            nc.vector.tensor_mul(t["v4"][:], t["v2"][:], t["v2"][:])._wait_ge(
                sa, sq_tick[k - 1]
            ).then_inc(sv, 1)
            nc.vector.tensor_mul(t["v6"][:], t["v4"][:], t["v2"][:])._wait_ge(
                sv, 2 * c + 1
            ).then_inc(sv, 1)

        # ---- sharp on sync (HWDGE), smooth on gpsimd (SWDGE)
        off = 0
        for c, ch in enumerate(CHUNKS_PE):
            t = tiles[c]
            sl = slice(off, off + ch)
            off += ch
            first = nc.sync.dma_start(sharp[0, :, sl], t["v6"][:])
            first._wait_ge(sv, 2 * c + 2)
            first.then_inc(ss, 16)
            for b in range(1, B_LOC):
                nc.sync.dma_start(sharp[b, :, sl], t["v6"][:]).then_inc(ss, 16)
            first = nc.gpsimd.dma_start(smooth[0, :, sl], t["rc"][:])
            first._wait_ge(sa, exp_tick[c])
            first.then_inc(sg, 16)
            for b in range(1, B_LOC):
                nc.gpsimd.dma_start(smooth[b, :, sl], t["rc"][:]).then_inc(sg, 16)

        # retire: all output DMAs complete (two standalone single waits)
        n_out = 16 * B_LOC * len(CHUNKS_PE)
        nc.sync.wait_ge(ss, n_out)
        nc.sync.wait_ge(sg, n_out)
    return nc


def kernel(gridx, gridy, gridz, mode, batchsize):
    _ensure_path()
    global _NC, LAST_RESULTS
    from concourse.bass_utils import run_bass_kernel_spmd

    m = int(mode)
    bsz = int(batchsize)
    assert m == MODE and bsz == BATCH, (m, bsz)

    gridx = np.asarray(gridx, np.float32)
    gridy = np.asarray(gridy, np.float32)
    gridz = np.asarray(gridz, np.float32)

    def cc(g):
        # f32 throughout, matching the f32 reference
        return (np.float32(-2.0) * np.cos(np.float32(2.0 * np.pi) * g)
                + np.float32(2.0))

    ccx = cc(np.concatenate([gridx[:m], gridx[-m:]]))   # [128]
    ccy = cc(np.concatenate([gridy[:m], gridy[-m:]]))   # [128]
    ccz = cc(gridz[:m])                                 # [64]

    byz = (ccy[:, None] + ccz[None, :]).reshape(-1).astype(np.float32)   # [8192]
    biasx = (np.float32(ALPHA) * ccx + np.float32(GAMMA)).astype(np.float32)  # [128]

    if _NC is None:
        _NC = {"pe": _build_nc_pe, "raw": _build_nc_raw, "tile": _build_nc}[IMPL]()

    if IMPL == "pe":
        rhs3 = (np.float32(ALPHA) * byz).astype(np.float16)[None, :]    # [1, 8192]
        ones1 = np.ones((1, TWO_M), np.float16)
        in_map = {"rhs3": rhs3, "ones1": ones1, "biasx": biasx}
    else:
        in_map = {"byz": byz, "biasx": biasx}
    in_maps = [dict(in_map) for _ in range(N_CORES)]
    res = run_bass_kernel_spmd(_NC, in_maps, core_ids=list(range(N_CORES)))
    LAST_RESULTS = res

    sharp = np.concatenate(
        [np.asarray(r["sharp"]).astype(np.float32).reshape(B_LOC, 1, TWO_M, TWO_M, MODE)
         for r in res.results], axis=0
    )
    smooth = np.concatenate(
        [np.asarray(r["smooth"]).astype(np.float32).reshape(B_LOC, 1, TWO_M, TWO_M, MODE)
         for r in res.results], axis=0
    )
    return (smooth, sharp)
